# revision 50
# baseline (speedup 1.0000x reference)
# Bass/Tile TRN2 kernel for nn_BlqSSM (Mamba2-SSD-style block with depthwise
# 3x3 conv, non-causal linear attention, LayerNorm gate, out-projection).
#
# Sharding: data-parallel over batch — 8 batches on 8 NeuronCores, weights
# replicated, no collectives. Each core computes one full batch element.
#
# v6 (~151us, from the 157-158us v2 baseline):
#   - startup: X loaded as contiguous row-slices with a small leading [0:512]
#     block so the first in_proj matmul fires as soon as ~0.6MB lands; W_in
#     split so the phase-1 slice (BC+dt cols) leads the sync queue; the conv
#     diag matrix (884KB of ~zeros) is built on-chip by the Pool engine from
#     [128, 27] coefficients x a shipped 128x128 identity.
#   - phase 1: dt in_proj -> XBAR transpose -> dA chain (one contiguous ACT
#     block, 3 act-table loads total); BC in_proj into padded conv buffer;
#     conv(BC) as 9 diag-weight PSUM-accumulated matmuls; SiLU fused into the
#     PSUM evacuation; B^T via XBAR.
#   - phase 2: xv in_proj + conv + V^T via XBAR (sync/scalar queues) + KV
#     accumulation (one PSUM tile over 32 chunks); z in_proj is interleaved
#     here too (PE+evac slack), landing channel-major bf16 in SBUF so the
#     phase-3 gate can run on the otherwise-idle Pool engine.
#   - phase 3 per-quad pipeline (q = 512 cols): y = C@KV (PE) -> D-skip +
#     bf16 downcast (DVE, frees PSUM) -> bn_stats x4 (DVE) -> stat merges
#     (Pool) -> rstd (ACT) -> yn = ysum*rstd + nmr (ACT x4) -> XBAR to
#     channel-major slab (sync) -> gate vs channel-major z (one Pool op) ->
#     out_proj (PE) -> single [128, 2, 512] PSUM evac to bf16 alternating
#     DVE/ACT -> output DMA (scalar queue). Output is bf16, converted to f32
#     on the host (rel-err cost ~1e-4). Gate emission trails its quad by one
#     iteration so no engine queue head-of-line-blocks on the XBAR chain.
#
# Known issue (pre-existing): bursty, environment-correlated per-core output
# corruption upstream of KV (~0-10%% of cores in bad minutes-long windows,
# odd cores dominate). See memory blqssm-flaky-correctness.
import os
import sys

for _p in ("/opt/trn_rl_repo", os.path.expanduser("~/.axon_site/_ro/trn_rl_repo")):
    if os.path.isdir(_p) and _p not in sys.path:
        sys.path.insert(0, _p)

import numpy as np
import ml_dtypes

import concourse.bass as bass
import concourse.mybir as mybir
import concourse.tile as tile
from concourse import bacc
from concourse.bass_utils import run_bass_kernel_spmd

F32 = mybir.dt.float32
BF16 = mybir.dt.bfloat16
AF = mybir.ActivationFunctionType
ALU = mybir.AluOpType

DIM = 256
DSTATE = 64
NHEADS = 8
HEAD_DIM = DIM // NHEADS          # 32
CONV_DIM = DIM + 2 * DSTATE       # 384
D_IN_PROJ = 2 * DIM + 2 * DSTATE + NHEADS  # 648
LN_EPS = 1e-5
B_, H_, W_ = 8, 64, 64
L = H_ * W_                       # 4096
NCORES = 8
NCH = 8                           # 512-wide column chunks of L
LCH = 32                          # 128-wide column chunks of L
PADW = H_ + 2                     # 66

BF = ml_dtypes.bfloat16

DUMP_MODE = 0  # 0=none 1=kv-early 4=bisect dumps (probe only)


def _build(flags, reps=1):
    generic_d = flags["generic_d"]
    has_lnw = flags["has_lnw"]
    has_lnb = flags["has_lnb"]

    nc = bacc.Bacc("TRN2", target_bir_lowering=False, debug=False, num_devices=NCORES)

    xd = nc.dram_tensor("xb", [DIM, L], BF16, kind="ExternalInput")
    w_ph1 = nc.dram_tensor("w_ph1", [DIM, 128 + NHEADS], BF16, kind="ExternalInput")
    w_ph2 = nc.dram_tensor("w_ph2", [DIM, DIM], BF16, kind="ExternalInput")
    w_z = nc.dram_tensor("w_z", [DIM, DIM], BF16, kind="ExternalInput")
    w_out = nc.dram_tensor("w_out", [DIM, DIM], BF16, kind="ExternalInput")
    identd = nc.dram_tensor("identd", [128, 128], BF16, kind="ExternalInput")
    convw = nc.dram_tensor("convw", [128, 27], F32, kind="ExternalInput")
    dtbias = nc.dram_tensor("dtbias", [NHEADS, 1], F32, kind="ExternalInput")
    expa = nc.dram_tensor("expa", [NHEADS, 1], F32, kind="ExternalInput")
    convb = nc.dram_tensor("convb", [128, 3], F32, kind="ExternalInput")
    dexpand = nc.dram_tensor("dexpand", [1, DIM], F32, kind="ExternalInput")
    lnwv = nc.dram_tensor("lnwv", [DIM], F32, kind="ExternalInput")
    lnbv = nc.dram_tensor("lnbv", [DIM], F32, kind="ExternalInput")
    outd = nc.dram_tensor("outb", [DIM, L], BF16, kind="ExternalOutput")
    if DUMP_MODE == 4:
        dbg = {"mode": 4,
               "d_kv": nc.dram_tensor("d_kv", [DSTATE, DIM], BF16, kind="ExternalOutput"),
               "d_Bt": nc.dram_tensor("d_Bt", [128, LCH, DSTATE], BF16, kind="ExternalOutput"),
               "d_Vt": nc.dram_tensor("d_Vt", [128, LCH, DIM], BF16, kind="ExternalOutput"),
               "d_dA": nc.dram_tensor("d_dA", [128, LCH, NHEADS], BF16, kind="ExternalOutput")}
    elif DUMP_MODE == 1:
        dbg = {"mode": 1,
               "d_kv": nc.dram_tensor("d_kv", [DSTATE, DIM], BF16, kind="ExternalOutput")}
    else:
        dbg = None

    with tile.TileContext(nc) as tc:
        _emit(nc, tc, locals(), generic_d, has_lnw, has_lnb, reps)
    nc.compile()
    return nc


def _emit(nc, tc, t, generic_d, has_lnw, has_lnb, reps=1):
    from contextlib import ExitStack

    xd, w_ph1, w_ph2, w_z, w_out = t["xd"], t["w_ph1"], t["w_ph2"], t["w_z"], t["w_out"]
    identd, convw, dtbias, expa = t["identd"], t["convw"], t["dtbias"], t["expa"]
    convb, dexpand, lnwv, lnbv, outd = t["convb"], t["dexpand"], t["lnwv"], t["lnbv"], t["outd"]
    dbg = t.get("dbg")

    with ExitStack() as ctx:
        const = ctx.enter_context(tc.tile_pool(name="const", bufs=1))
        big = ctx.enter_context(tc.tile_pool(name="big", bufs=1))

        # ---- weights + X, ordered so the first matmuls unblock earliest.
        # X is loaded as contiguous [128, 2048] row-slices (4KB/row
        # descriptors); the phase-1 W_in slice (BC cols + dt cols) leads the
        # sync queue so dt/BC matmuls can start ~2us in. ----
        wxb1 = const.tile([128, 2, 128 + NHEADS], BF16)
        nc.sync.dma_start(wxb1[:], w_ph1.ap().rearrange("(t k) m -> k t m", k=128))
        X = big.tile([128, 2, L], BF16)
        for (lo, hi) in ((0, 512), (512, 1024), (1024, 2048), (2048, 4096)):
            nc.sync.dma_start(X[:, 0, lo:hi], xd.ap()[0:128, lo:hi])
            nc.scalar.dma_start(X[:, 1, lo:hi], xd.ap()[128:256, lo:hi])
        # conv diag build inputs (tiny), then the rest of the constants
        ident = const.tile([128, 128], BF16)
        nc.sync.dma_start(ident[:], identd.ap())
        cw = const.tile([128, 27], F32)
        nc.sync.dma_start(cw[:], convw.ap())
        cb = const.tile([128, 3], F32)
        nc.sync.dma_start(cb[:], convb.ap())
        dtb_t = const.tile([128, NHEADS], F32)
        nc.sync.dma_start(dtb_t[:], dtbias.ap().rearrange("a b -> b a").partition_broadcast(128))
        ea_t = const.tile([128, NHEADS], F32)
        nc.sync.dma_start(ea_t[:], expa.ap().rearrange("a b -> b a").partition_broadcast(128))
        eps_t = const.tile([128, 1], F32)
        nc.vector.memset(eps_t[:], LN_EPS)
        wxb2 = const.tile([128, 2, DIM], BF16)
        nc.scalar.dma_start(wxb2[:], w_ph2.ap().rearrange("(t k) m -> k t m", k=128))
        wz = const.tile([128, 2, DIM], BF16)
        nc.scalar.dma_start(wz[:], w_z.ap().rearrange("(t k) m -> k t m", k=128))
        wo = const.tile([128, 2, DIM], BF16)
        nc.scalar.dma_start(wo[:], w_out.ap().rearrange("(t k) m -> k t m", k=128))
        if generic_d:
            dm1_bc = const.tile([128, DIM], F32)
            nc.sync.dma_start(dm1_bc[:], dexpand.ap().partition_broadcast(128))
        if has_lnw:
            lnw_bc = const.tile([128, DIM], F32)
            nc.sync.dma_start(lnw_bc[:], lnwv.ap().unsqueeze(0).partition_broadcast(128))
        if has_lnb:
            lnb_bc = const.tile([128, DIM], F32)
            nc.sync.dma_start(lnb_bc[:], lnbv.ap().unsqueeze(0).partition_broadcast(128))

        # ---- persistent activations ----
        pads = [big.tile([128, PADW, PADW], BF16, name=f"pad{g}") for g in range(3)]
        V0 = big.tile([128, L], BF16)
        V1 = big.tile([128, L], BF16)
        Vg = [V0, V1]
        Vt = big.tile([128, LCH, DIM], BF16)      # V^T, L-major, per-chunk
        BC = big.tile([128, L], BF16)             # B at parts 0:64, C at 64:128
        Bt = big.tile([128, LCH, DSTATE], BF16)   # B^T, L-major
        dtraw = big.tile([16, L], BF16)
        dtt = big.tile([128, LCH, 16], BF16)
        dAsc = big.tile([128, LCH, NHEADS], BF16)
        kv_sb = big.tile([128, DIM], BF16)        # KV parked at partitions 64:128
        zcm = big.tile([128, 2, L], BF16)         # z, channel-major [c128, g, l]
        cdg = big.tile([128, 27, 128], BF16)      # conv diag, built on-chip

        # build the 27 conv diagonals (g=2 taps first: phase-1 conv uses them)
        for ti in list(range(18, 27)) + list(range(18)):
            nc.gpsimd.tensor_mul(cdg[:, ti, :], ident[:],
                                 cw[:, ti:ti + 1].to_broadcast([128, 128]))

        for _rep in range(reps):
            # border-only zeroing of the conv pads (interior fully overwritten)
            for g in range(3):
                p = pads[g][:]
                nc.gpsimd.memset(p[:, 0:1, :], 0.0)
                nc.gpsimd.memset(p[:, PADW - 1:PADW, :], 0.0)
                nc.gpsimd.memset(p[:, 1:PADW - 1, 0:1], 0.0)
                nc.gpsimd.memset(p[:, 1:PADW - 1, PADW - 1:PADW], 0.0)
            if _rep == 0:
                nc.gpsimd.memset(dtraw[:], 0.0)

            with (
                tc.tile_pool(name="pconv", bufs=2, space="PSUM") as pconv,
                tc.tile_pool(name="p1a", bufs=3, space="PSUM") as p1a,
                tc.tile_pool(name="pdt", bufs=2, space="PSUM") as pdt,
                tc.tile_pool(name="pkv", bufs=1, space="PSUM") as pkv,
                tc.tile_pool(name="daf", bufs=1) as daf_pool,
                tc.tile_pool(name="sbt", bufs=3) as sbt,
            ):
                # ---- Phase 1: dt in_proj + dA chain; BC in_proj; conv(BC) ----
                if True:
                    # dt / BC in_proj round-robin across PSUM pools so each
                    # evacuation has slack (keeps PE dense)
                    def dt_mm(n):
                        ps = pdt.tile([NHEADS, 512], F32)
                        for k in range(2):
                            nc.tensor.matmul(ps[:], wxb1[:, k, 128:],
                                             X[:, k, n * 512:(n + 1) * 512],
                                             start=(k == 0), stop=(k == 1))
                        nc.vector.tensor_copy(dtraw[0:NHEADS, n * 512:(n + 1) * 512], ps[:])
                    def bc_mm(n):
                        ps = p1a.tile([128, 512], F32)
                        for k in range(2):
                            nc.tensor.matmul(ps[:], wxb1[:, k, 0:128],
                                             X[:, k, n * 512:(n + 1) * 512],
                                             start=(k == 0), stop=(k == 1))
                        r0 = n * 8
                        nc.vector.tensor_copy(pads[2][:, r0 + 1:r0 + 9, 1:1 + W_],
                                              ps[:].rearrange("p (r w) -> p r w", w=W_))

                    for n in range(NCH):
                        dt_mm(n)
                        bc_mm(n)
                    nc.sync.dma_start_transpose(dtt[:], dtraw[:])
                    # dA = Ln(Exp(dt + bias) + 1) * exp(A_log), one contiguous
                    # ACT block so only one Exp/Ln table set load happens.
                    dAf = daf_pool.tile([128, LCH, NHEADS], F32, tag="dAf")
                    nc.vector.tensor_add(
                        dAf[:], dtt[:, :, 0:NHEADS],
                        dtb_t[:].unsqueeze(1).to_broadcast([128, LCH, NHEADS]))
                    nc.scalar.activation(dAf[:], dAf[:], AF.Exp)
                    nc.scalar.activation(dAf[:], dAf[:], AF.Ln, bias=1.0)
                    nc.vector.tensor_mul(
                        dAsc[:], dAf[:],
                        ea_t[:].unsqueeze(1).to_broadcast([128, LCH, NHEADS]))
                    if dbg is not None and dbg.get("mode") == 4:
                        nc.gpsimd.dma_start(dbg["d_dA"].ap(), dAsc[:])
                    # conv(BC) + SiLU evacuation
                    for n in range(NCH):
                        pc = pconv.tile([128, 512], F32, tag="pc")
                        r0 = n * 8
                        for ti in range(9):
                            dy, dx = ti // 3, ti % 3
                            nc.tensor.matmul(
                                pc[:].rearrange("p (r w) -> p r w", w=W_),
                                cdg[:, 18 + ti, :],
                                pads[2][:, r0 + dy:r0 + dy + 8, dx:dx + W_],
                                start=(ti == 0), stop=(ti == 8))
                        nc.scalar.activation(BC[:, n * 512:(n + 1) * 512], pc[:],
                                             AF.Silu, bias=cb[:, 2:3])
                    # B^T via XBAR (overlaps phase 2 compute)
                    nc.sync.dma_start_transpose(Bt[:], BC[0:DSTATE, :])
                    if dbg is not None and dbg.get("mode") == 4:
                        nc.gpsimd.dma_start(dbg["d_Bt"].ap(), Bt[:])

                # ---- Phase 2: xv in_proj + conv + V^T via XBAR + KV ----
                if True:
                    kvp = pkv.tile([DSTATE, DIM], F32)

                    def inproj_xv(n):
                        for g in range(2):
                            ps = p1a.tile([128, 512], F32)
                            for k in range(2):
                                nc.tensor.matmul(ps[:], wxb2[:, k, g * 128:(g + 1) * 128],
                                                 X[:, k, n * 512:(n + 1) * 512],
                                                 start=(k == 0), stop=(k == 1))
                            r0 = n * 8
                            dst = pads[g][:, r0 + 1:r0 + 9, 1:1 + W_]
                            src2 = ps[:].rearrange("p (r w) -> p r w", w=W_)
                            if g == 1:
                                nc.vector.tensor_copy(dst, src2)
                            else:
                                nc.scalar.copy(dst, src2)

                    def conv_v(n):
                        for g in range(2):
                            pc = pconv.tile([128, 512], F32, tag="pc")
                            r0 = n * 8
                            for ti in range(9):
                                dy, dx = ti // 3, ti % 3
                                nc.tensor.matmul(
                                    pc[:].rearrange("p (r w) -> p r w", w=W_),
                                    cdg[:, g * 9 + ti, :],
                                    pads[g][:, r0 + dy:r0 + dy + 8, dx:dx + W_],
                                    start=(ti == 0), stop=(ti == 8))
                            nc.scalar.activation(Vg[g][:, n * 512:(n + 1) * 512], pc[:],
                                                 AF.Silu, bias=cb[:, g:g + 1])
                            eng = nc.sync if g == 0 else nc.scalar
                            eng.dma_start_transpose(
                                Vt[:, 4 * n:4 * n + 4, g * 128:(g + 1) * 128],
                                Vg[g][:, n * 512:(n + 1) * 512])
                        if dbg is not None and dbg.get("mode") == 4:
                            nc.gpsimd.dma_start(dbg["d_Vt"].ap()[:, 4 * n:4 * n + 4, :],
                                                Vt[:, 4 * n:4 * n + 4, :])

                    def z_mm(n):
                        # z in_proj in phase-2 slack; evacs land channel-major
                        # in SBUF so the tail gate can run on the Pool engine
                        for g in range(2):
                            ps = p1a.tile([128, 512], F32)
                            for k in range(2):
                                nc.tensor.matmul(ps[:], wz[:, k, g * 128:(g + 1) * 128],
                                                 X[:, k, n * 512:(n + 1) * 512],
                                                 start=(k == 0), stop=(k == 1))
                            if g == 0:
                                nc.vector.tensor_copy(zcm[:, g, n * 512:(n + 1) * 512], ps[:])
                            else:
                                nc.scalar.copy(zcm[:, g, n * 512:(n + 1) * 512], ps[:])

                    def ssd(n):
                        for i in (4 * n, 4 * n + 2):
                            vst = sbt.tile([128, 2, DIM], BF16, tag="vst")
                            nc.vector.tensor_mul(
                                vst[:].rearrange("p c (h q) -> p c h q", h=NHEADS),
                                Vt[:, i:i + 2, :].rearrange("p c (h q) -> p c h q", h=NHEADS),
                                dAsc[:, i:i + 2, :].unsqueeze(3).to_broadcast(
                                    [128, 2, NHEADS, HEAD_DIM]),
                            )
                            for j in range(2):
                                nc.tensor.matmul(kvp[:], Bt[:, i + j, :], vst[:, j, :],
                                                 start=(i + j == 0), stop=(i + j == LCH - 1))

                    for n in range(NCH + 2):
                        if n < NCH:
                            inproj_xv(n)
                        if 1 <= n < NCH + 1:
                            conv_v(n - 1)
                        if n >= 2:
                            ssd(n - 2)
                            z_mm(n - 2)
                    z_mm(NCH - 2)
                    z_mm(NCH - 1)
                    nc.vector.tensor_copy(kv_sb[DSTATE:2 * DSTATE, :], kvp[:])
                    if dbg is not None and dbg.get("mode") in (1, 4):
                        nc.gpsimd.dma_start(dbg["d_kv"].ap(), kv_sb[DSTATE:2 * DSTATE, :])

            # ---- Phase 3: per-quad pipeline: y, D-skip, LN, XBAR, gate,
            # z in_proj (channel-major, into dead pads), out_proj ----
            with (
                tc.tile_pool(name="pyp", bufs=2, space="PSUM") as pyp,
                tc.tile_pool(name="pzo", bufs=2, space="PSUM") as pzo,
                tc.tile_pool(name="s6", bufs=8) as s6,
                tc.tile_pool(name="s6b", bufs=8) as s6b,
                tc.tile_pool(name="s6g", bufs=12) as s6g,
            ):
                outr = outd.ap().rearrange("(m k) l -> k m l", k=128)
                slabs = [None] * NCH

                def y_quad(q):
                    ypr = pyp.tile([128, 4, DIM], F32, tag="ypr")
                    for j in range(4):
                        i = 4 * q + j
                        sl = slice(i * 128, (i + 1) * 128)
                        nc.tensor.matmul(ypr[:, j, :], BC[DSTATE:2 * DSTATE, sl],
                                         kv_sb[DSTATE:2 * DSTATE, :], start=True, stop=True)
                    # D-skip: single DVE op PSUM->SBUF frees the PSUM tile;
                    # bf16 result halves the DVE time of the bn_stats after
                    ysum = s6b.tile([128, 4, DIM], BF16, tag="ysum")
                    if generic_d:
                        vtmp = s6b.tile([128, 4, DIM], F32, tag="vtmp")
                        nc.vector.tensor_mul(
                            vtmp[:], Vt[:, 4 * q:4 * q + 4, :],
                            dm1_bc[:].unsqueeze(1).to_broadcast([128, 4, DIM]))
                        nc.vector.tensor_add(ysum[:], ypr[:], vtmp[:])
                    else:
                        nc.vector.tensor_add(ysum[:], ypr[:], Vt[:, 4 * q:4 * q + 4, :])
                    st = s6.tile([128, 4, 6], F32, tag="st")
                    for j in range(4):
                        nc.vector.bn_stats(st[:, j, :], ysum[:, j, :])
                    return ysum, st

                def ln_quad(q, ysum, st):
                    # bn_stats groups are the even/odd lanes of each 256-chunk
                    # (st per chunk = [n0, m0, M2_0, n1, m1, M2_1]); merged
                    # with tiny Pool ops:
                    #   mean = (m0+m1)/2
                    #   var  = (M2_0+M2_1)/256 + (m0-m1)^2/4
                    stm = st[:].rearrange("p j (g s) -> p j g s", g=2)
                    # variance merges on DVE: same queue as bn_stats, so the
                    # rstd input reaches ACT with a single cross-engine hop
                    msum = s6.tile([128, 4], F32, tag="msum")
                    nc.vector.tensor_add(msum[:], stm[:, :, 0, 1], stm[:, :, 1, 1])
                    mdif = s6.tile([128, 4], F32, tag="mdif")
                    nc.vector.tensor_sub(mdif[:], stm[:, :, 0, 1], stm[:, :, 1, 1])
                    m2s = s6.tile([128, 4], F32, tag="m2s")
                    nc.vector.tensor_add(m2s[:], stm[:, :, 0, 2], stm[:, :, 1, 2])
                    md2 = s6.tile([128, 4], F32, tag="md2")
                    nc.vector.tensor_mul(md2[:], mdif[:], mdif[:])
                    # var*256 = m2s + 64*md2 -> rstd = rsqrt((m2s+64*md2)/256+eps)
                    var256 = s6.tile([128, 4], F32, tag="var256")
                    nc.vector.scalar_tensor_tensor(var256[:], md2[:], 64.0, m2s[:],
                                                   ALU.mult, ALU.add)
                    rstd = s6.tile([128, 4], F32, tag="rstd")
                    nc.scalar.activation(rstd[:], var256[:], AF.Abs_reciprocal_sqrt,
                                         bias=eps_t[:], scale=1.0 / DIM)
                    nmr = s6.tile([128, 4], F32, tag="nmr")
                    nc.gpsimd.tensor_mul(nmr[:], msum[:], rstd[:])
                    nc.gpsimd.tensor_scalar_mul(nmr[:], nmr[:], -0.5)
                    # g-major layout [c? no: l, g, j, c128] so the post-XBAR slab
                    # has each channel group contiguous (fast gate + out stream)
                    yn = s6b.tile([128, 2, 4, 128], BF16, tag="yn")
                    for j in range(4):
                        nc.scalar.activation(
                            yn[:, :, j, :],
                            ysum[:, j, :].rearrange("p (g c) -> p g c", g=2),
                            AF.Identity,
                            bias=nmr[:, j:j + 1], scale=rstd[:, j:j + 1])
                    if has_lnw:
                        nc.vector.tensor_mul(
                            yn[:], yn[:],
                            lnw_bc[:].rearrange("p (g c) -> p g c", g=2)
                            .unsqueeze(2).to_broadcast([128, 2, 4, 128]))
                    if has_lnb:
                        nc.vector.tensor_add(
                            yn[:], yn[:],
                            lnb_bc[:].rearrange("p (g c) -> p g c", g=2)
                            .unsqueeze(2).to_broadcast([128, 2, 4, 128]))
                    return yn

                def tr_quad(q, yn):
                    # slab [c128, g, j, l]: channel-major quad, g contiguous
                    slab = s6g.tile([128, 2, 4, 128], BF16, tag="yzt")
                    nc.sync.dma_start_transpose(
                        slab[:], yn[:].rearrange("a b c d -> a (b c d)"))
                    slabs[q] = slab

                def gate_quad(q):
                    # fully contiguous: slab [c128, g, j, l] vs z [c128, g, 512]
                    slab = slabs[q]
                    zv = zcm[:, :, 512 * q:512 * (q + 1)].rearrange(
                        "p g (n l) -> p g n l", l=128)
                    nc.gpsimd.tensor_mul(slab[:], slab[:], zv)

                def out_quad(q):
                    slab = slabs[q]
                    ob = s6g.tile([128, 2, 512], BF16, tag="ob")
                    po = pzo.tile([128, 2, 512], F32, tag="ops")
                    for mo in range(2):
                        for k in range(2):
                            nc.tensor.matmul(po[:, mo, :], wo[:, k, mo * 128:(mo + 1) * 128],
                                             slab[:, k, :, :], start=(k == 0), stop=(k == 1))
                    # one big evacuation, alternating engines per quad
                    if q % 2 == 0:
                        nc.vector.tensor_copy(ob[:], po[:])
                    else:
                        nc.scalar.copy(ob[:], po[:])
                    nc.scalar.dma_start(outr[:, :, q * 512:(q + 1) * 512], ob[:])

                # software pipeline: PE order y(q), z(q), out(q-2) so out_proj
                # trails the yn/XBAR/gate chain by ~2 quads of slack
                for q in range(NCH + 4):
                    if q < NCH:
                        ysum, st = y_quad(q)
                    if 2 <= q < NCH + 2:
                        gate_quad(q - 2)
                    if q < NCH:
                        yn = ln_quad(q, ysum, st)
                        tr_quad(q, yn)
                    if q >= 4:
                        out_quad(q - 4)


_CACHE = {}


def _prep(W_in, conv_w, conv_b, A_log, dt_bias, D, ln_w, ln_b, W_out):
    W_in = np.asarray(W_in, np.float32)
    conv_w = np.asarray(conv_w, np.float32)
    conv_b = np.asarray(conv_b, np.float32)
    A_log = np.asarray(A_log, np.float32)
    dt_bias = np.asarray(dt_bias, np.float32)
    D = np.asarray(D, np.float32)
    ln_w = np.asarray(ln_w, np.float32)
    ln_b = np.asarray(ln_b, np.float32)
    W_out = np.asarray(W_out, np.float32)

    WinT = np.ascontiguousarray(W_in.T)                       # [256, 648]
    # phase-1 slice: BC cols (W_in rows 512:640) + dt cols (640:648)
    w_ph1 = np.ascontiguousarray(WinT[:, 2 * DIM:]).astype(BF)   # [256, 136]
    w_ph2 = np.ascontiguousarray(WinT[:, DIM:2 * DIM]).astype(BF)  # [256, 256] xv
    w_z = np.ascontiguousarray(WinT[:, :DIM]).astype(BF)      # [256, 256]
    w_out_t = np.ascontiguousarray(W_out.T).astype(BF)        # [256, 256]

    cwm = np.zeros((128, 27), np.float32)
    idx = np.arange(128)
    for g in range(3):
        for t in range(9):
            cwm[:, g * 9 + t] = conv_w[g * 128 + idx, 0, t // 3, t % 3]

    cbm = np.zeros((128, 3), np.float32)
    for g in range(3):
        cbm[:, g] = conv_b[g * 128:(g + 1) * 128]

    flags = {
        "generic_d": not np.allclose(D, 1.0),
        "has_lnw": not np.allclose(ln_w, 1.0),
        "has_lnb": not np.allclose(ln_b, 0.0),
    }
    shared = dict(
        w_ph1=w_ph1, w_ph2=w_ph2, w_z=w_z, w_out=w_out_t,
        identd=np.eye(128, dtype=np.float32).astype(BF), convw=cwm,
        dtbias=dt_bias.reshape(NHEADS, 1).astype(np.float32),
        expa=np.exp(A_log).reshape(NHEADS, 1).astype(np.float32),
        convb=cbm, dexpand=np.repeat(D, HEAD_DIM).reshape(1, DIM).astype(np.float32),
        lnwv=ln_w, lnbv=ln_b,
    )
    return flags, shared


def _get_nc(flags, reps=1):
    key = (tuple(sorted(flags.items())), reps, DUMP_MODE)
    if key not in _CACHE:
        _CACHE[key] = _build(flags, reps)
    return _CACHE[key]


def kernel(x, W_in, conv_w, conv_b, A_log, dt_bias, D, ln_w, ln_b, W_out,
           _trace=False):
    x = np.asarray(x, np.float32)
    flags, shared = _prep(W_in, conv_w, conv_b, A_log, dt_bias, D, ln_w, ln_b, W_out)
    nc = _get_nc(flags)
    xb = x.reshape(B_, DIM, L).astype(BF)
    in_maps = [dict(xb=np.ascontiguousarray(xb[b]), **shared) for b in range(B_)]
    res = run_bass_kernel_spmd(nc, in_maps, core_ids=list(range(NCORES)), trace=_trace)
    out = np.stack([np.asarray(res.results[b]["outb"], np.float32) for b in range(B_)])
    out = out.reshape(B_, DIM, H_, W_)
    if _trace:
        return out, res
    return out


# revision 51
# speedup vs baseline: 1.0070x; 1.0070x over previous
# Bass/Tile TRN2 kernel for nn_BlqSSM (Mamba2-SSD-style block with depthwise
# 3x3 conv, non-causal linear attention, LayerNorm gate, out-projection).
#
# Sharding: data-parallel over batch — 8 batches on 8 NeuronCores, weights
# replicated, no collectives. Each core computes one full batch element.
#
# v6 (~151us, from the 157-158us v2 baseline):
#   - startup: X loaded as contiguous row-slices with a small leading [0:512]
#     block so the first in_proj matmul fires as soon as ~0.6MB lands; W_in
#     split so the phase-1 slice (BC+dt cols) leads the sync queue; the conv
#     diag matrix (884KB of ~zeros) is built on-chip by the Pool engine from
#     [128, 27] coefficients x a shipped 128x128 identity.
#   - phase 1: dt in_proj -> XBAR transpose -> dA chain (one contiguous ACT
#     block, 3 act-table loads total); BC in_proj into padded conv buffer;
#     conv(BC) as 9 diag-weight PSUM-accumulated matmuls; SiLU fused into the
#     PSUM evacuation; B^T via XBAR.
#   - phase 2: xv in_proj + conv + V^T via XBAR (sync/scalar queues) + KV
#     accumulation (one PSUM tile over 32 chunks); z in_proj is interleaved
#     here too (PE+evac slack), landing channel-major bf16 in SBUF so the
#     phase-3 gate can run on the otherwise-idle Pool engine.
#   - phase 3 per-quad pipeline (q = 512 cols): y = C@KV (PE) -> D-skip +
#     bf16 downcast (DVE, frees PSUM) -> bn_stats x4 (DVE) -> stat merges
#     (Pool) -> rstd (ACT) -> yn = ysum*rstd + nmr (ACT x4) -> XBAR to
#     channel-major slab (sync) -> gate vs channel-major z (one Pool op) ->
#     out_proj (PE) -> single [128, 2, 512] PSUM evac to bf16 alternating
#     DVE/ACT -> output DMA (scalar queue). Output is bf16, converted to f32
#     on the host (rel-err cost ~1e-4). Gate emission trails its quad by one
#     iteration so no engine queue head-of-line-blocks on the XBAR chain.
#
# Known issue (pre-existing): bursty, environment-correlated per-core output
# corruption upstream of KV (~0-10%% of cores in bad minutes-long windows,
# odd cores dominate). See memory blqssm-flaky-correctness.
import os
import sys

for _p in ("/opt/trn_rl_repo", os.path.expanduser("~/.axon_site/_ro/trn_rl_repo")):
    if os.path.isdir(_p) and _p not in sys.path:
        sys.path.insert(0, _p)

import numpy as np
import ml_dtypes

import concourse.bass as bass
import concourse.mybir as mybir
import concourse.tile as tile
from concourse import bacc
from concourse.bass_utils import run_bass_kernel_spmd

F32 = mybir.dt.float32
BF16 = mybir.dt.bfloat16
AF = mybir.ActivationFunctionType
ALU = mybir.AluOpType

DIM = 256
DSTATE = 64
NHEADS = 8
HEAD_DIM = DIM // NHEADS          # 32
CONV_DIM = DIM + 2 * DSTATE       # 384
D_IN_PROJ = 2 * DIM + 2 * DSTATE + NHEADS  # 648
LN_EPS = 1e-5
B_, H_, W_ = 8, 64, 64
L = H_ * W_                       # 4096
NCORES = 8
NCH = 8                           # 512-wide column chunks of L
LCH = 32                          # 128-wide column chunks of L
PADW = H_ + 2                     # 66

BF = ml_dtypes.bfloat16

DUMP_MODE = 0  # 0=none 1=kv-early 4=bisect dumps (probe only)


def _build(flags, reps=1):
    generic_d = flags["generic_d"]
    has_lnw = flags["has_lnw"]
    has_lnb = flags["has_lnb"]

    nc = bacc.Bacc("TRN2", target_bir_lowering=False, debug=False, num_devices=NCORES)

    xd = nc.dram_tensor("xb", [DIM, L], BF16, kind="ExternalInput")
    w_ph1 = nc.dram_tensor("w_ph1", [DIM, 128 + NHEADS], BF16, kind="ExternalInput")
    w_ph2 = nc.dram_tensor("w_ph2", [DIM, DIM], BF16, kind="ExternalInput")
    w_z = nc.dram_tensor("w_z", [DIM, DIM], BF16, kind="ExternalInput")
    w_out = nc.dram_tensor("w_out", [DIM, DIM], BF16, kind="ExternalInput")
    identd = nc.dram_tensor("identd", [128, 128], BF16, kind="ExternalInput")
    convw = nc.dram_tensor("convw", [128, 27], F32, kind="ExternalInput")
    dtbias = nc.dram_tensor("dtbias", [NHEADS, 1], F32, kind="ExternalInput")
    expa = nc.dram_tensor("expa", [NHEADS, 1], F32, kind="ExternalInput")
    convb = nc.dram_tensor("convb", [128, 3], F32, kind="ExternalInput")
    dexpand = nc.dram_tensor("dexpand", [1, DIM], F32, kind="ExternalInput")
    lnwv = nc.dram_tensor("lnwv", [DIM], F32, kind="ExternalInput")
    lnbv = nc.dram_tensor("lnbv", [DIM], F32, kind="ExternalInput")
    outd = nc.dram_tensor("outb", [DIM, L], BF16, kind="ExternalOutput")
    if DUMP_MODE == 4:
        dbg = {"mode": 4,
               "d_kv": nc.dram_tensor("d_kv", [DSTATE, DIM], BF16, kind="ExternalOutput"),
               "d_Bt": nc.dram_tensor("d_Bt", [128, LCH, DSTATE], BF16, kind="ExternalOutput"),
               "d_Vt": nc.dram_tensor("d_Vt", [128, LCH, DIM], BF16, kind="ExternalOutput"),
               "d_dA": nc.dram_tensor("d_dA", [128, LCH, NHEADS], BF16, kind="ExternalOutput")}
    elif DUMP_MODE == 1:
        dbg = {"mode": 1,
               "d_kv": nc.dram_tensor("d_kv", [DSTATE, DIM], BF16, kind="ExternalOutput")}
    else:
        dbg = None

    with tile.TileContext(nc) as tc:
        _emit(nc, tc, locals(), generic_d, has_lnw, has_lnb, reps)
    nc.compile()
    return nc


def _emit(nc, tc, t, generic_d, has_lnw, has_lnb, reps=1):
    from contextlib import ExitStack

    xd, w_ph1, w_ph2, w_z, w_out = t["xd"], t["w_ph1"], t["w_ph2"], t["w_z"], t["w_out"]
    identd, convw, dtbias, expa = t["identd"], t["convw"], t["dtbias"], t["expa"]
    convb, dexpand, lnwv, lnbv, outd = t["convb"], t["dexpand"], t["lnwv"], t["lnbv"], t["outd"]
    dbg = t.get("dbg")

    with ExitStack() as ctx:
        const = ctx.enter_context(tc.tile_pool(name="const", bufs=1))
        big = ctx.enter_context(tc.tile_pool(name="big", bufs=1))

        # ---- weights + X, ordered so the first matmuls unblock earliest.
        # X is loaded as contiguous [128, 2048] row-slices (4KB/row
        # descriptors); the phase-1 W_in slice (BC cols + dt cols) leads the
        # sync queue so dt/BC matmuls can start ~2us in. ----
        wxb1 = const.tile([128, 2, 128 + NHEADS], BF16)
        nc.sync.dma_start(wxb1[:], w_ph1.ap().rearrange("(t k) m -> k t m", k=128))
        X = big.tile([128, 2, L], BF16)
        for (lo, hi) in ((0, 512), (512, 1024), (1024, 2048), (2048, 4096)):
            nc.sync.dma_start(X[:, 0, lo:hi], xd.ap()[0:128, lo:hi])
            nc.scalar.dma_start(X[:, 1, lo:hi], xd.ap()[128:256, lo:hi])
        # conv diag build inputs (tiny), then the rest of the constants
        ident = const.tile([128, 128], BF16)
        nc.sync.dma_start(ident[:], identd.ap())
        cw = const.tile([128, 27], F32)
        nc.sync.dma_start(cw[:], convw.ap())
        cb = const.tile([128, 3], F32)
        nc.sync.dma_start(cb[:], convb.ap())
        dtb_t = const.tile([128, NHEADS], F32)
        nc.sync.dma_start(dtb_t[:], dtbias.ap().rearrange("a b -> b a").partition_broadcast(128))
        ea_t = const.tile([128, NHEADS], F32)
        nc.sync.dma_start(ea_t[:], expa.ap().rearrange("a b -> b a").partition_broadcast(128))
        eps_t = const.tile([128, 1], F32)
        nc.vector.memset(eps_t[:], LN_EPS)
        wxb2 = const.tile([128, 2, DIM], BF16)
        nc.scalar.dma_start(wxb2[:], w_ph2.ap().rearrange("(t k) m -> k t m", k=128))
        wz = const.tile([128, 2, DIM], BF16)
        nc.scalar.dma_start(wz[:], w_z.ap().rearrange("(t k) m -> k t m", k=128))
        wo = const.tile([128, 2, DIM], BF16)
        nc.scalar.dma_start(wo[:], w_out.ap().rearrange("(t k) m -> k t m", k=128))
        if generic_d:
            dm1_bc = const.tile([128, DIM], F32)
            nc.sync.dma_start(dm1_bc[:], dexpand.ap().partition_broadcast(128))
        if has_lnw:
            lnw_bc = const.tile([128, DIM], F32)
            nc.sync.dma_start(lnw_bc[:], lnwv.ap().unsqueeze(0).partition_broadcast(128))
        if has_lnb:
            lnb_bc = const.tile([128, DIM], F32)
            nc.sync.dma_start(lnb_bc[:], lnbv.ap().unsqueeze(0).partition_broadcast(128))

        # ---- persistent activations ----
        pads = [big.tile([128, PADW, PADW], BF16, name=f"pad{g}") for g in range(3)]
        V0 = big.tile([128, L], BF16)
        V1 = big.tile([128, L], BF16)
        Vg = [V0, V1]
        Vt = big.tile([128, LCH, DIM], BF16)      # V^T, L-major, per-chunk
        BC = big.tile([128, L], BF16)             # B at parts 0:64, C at 64:128
        Bt = big.tile([128, LCH, DSTATE], BF16)   # B^T, L-major
        dtraw = big.tile([16, L], BF16)
        dtt = big.tile([128, LCH, 16], BF16)
        dAsc = big.tile([128, LCH, NHEADS], BF16)
        kv_sb = big.tile([128, DIM], BF16)        # KV parked at partitions 64:128
        zcm = big.tile([128, 2, L], BF16)         # z, channel-major [c128, g, l]
        cdg = big.tile([128, 27, 128], BF16)      # conv diag, built on-chip

        # build the 27 conv diagonals (g=2 taps first: phase-1 conv uses them)
        for ti in list(range(18, 27)) + list(range(18)):
            nc.gpsimd.tensor_mul(cdg[:, ti, :], ident[:],
                                 cw[:, ti:ti + 1].to_broadcast([128, 128]))

        for _rep in range(reps):
            # border-only zeroing of the conv pads (interior fully overwritten)
            for g in range(3):
                p = pads[g][:]
                nc.gpsimd.memset(p[:, 0:1, :], 0.0)
                nc.gpsimd.memset(p[:, PADW - 1:PADW, :], 0.0)
                nc.gpsimd.memset(p[:, 1:PADW - 1, 0:1], 0.0)
                nc.gpsimd.memset(p[:, 1:PADW - 1, PADW - 1:PADW], 0.0)
            if _rep == 0:
                nc.gpsimd.memset(dtraw[:], 0.0)

            with (
                tc.tile_pool(name="pconv", bufs=2, space="PSUM") as pconv,
                tc.tile_pool(name="p1a", bufs=3, space="PSUM") as p1a,
                tc.tile_pool(name="pdt", bufs=2, space="PSUM") as pdt,
                tc.tile_pool(name="pkv", bufs=1, space="PSUM") as pkv,
                tc.tile_pool(name="daf", bufs=1) as daf_pool,
                tc.tile_pool(name="sbt", bufs=3) as sbt,
            ):
                # ---- Phase 1: dt in_proj + dA chain; BC in_proj; conv(BC) ----
                if True:
                    # dt / BC in_proj round-robin across PSUM pools so each
                    # evacuation has slack (keeps PE dense)
                    def dt_mm(n):
                        ps = pdt.tile([NHEADS, 512], F32)
                        for k in range(2):
                            nc.tensor.matmul(ps[:], wxb1[:, k, 128:],
                                             X[:, k, n * 512:(n + 1) * 512],
                                             start=(k == 0), stop=(k == 1))
                        nc.vector.tensor_copy(dtraw[0:NHEADS, n * 512:(n + 1) * 512], ps[:])
                    def bc_mm(n):
                        ps = p1a.tile([128, 512], F32)
                        for k in range(2):
                            nc.tensor.matmul(ps[:], wxb1[:, k, 0:128],
                                             X[:, k, n * 512:(n + 1) * 512],
                                             start=(k == 0), stop=(k == 1))
                        r0 = n * 8
                        nc.vector.tensor_copy(pads[2][:, r0 + 1:r0 + 9, 1:1 + W_],
                                              ps[:].rearrange("p (r w) -> p r w", w=W_))

                    for n in range(NCH):
                        dt_mm(n)
                        bc_mm(n)
                    nc.sync.dma_start_transpose(dtt[:], dtraw[:])
                    # dA = Ln(Exp(dt + bias) + 1) * exp(A_log), one contiguous
                    # ACT block so only one Exp/Ln table set load happens.
                    dAf = daf_pool.tile([128, LCH, NHEADS], F32, tag="dAf")
                    nc.vector.tensor_add(
                        dAf[:], dtt[:, :, 0:NHEADS],
                        dtb_t[:].unsqueeze(1).to_broadcast([128, LCH, NHEADS]))
                    nc.scalar.activation(dAf[:], dAf[:], AF.Exp)
                    nc.scalar.activation(dAf[:], dAf[:], AF.Ln, bias=1.0)
                    nc.vector.tensor_mul(
                        dAsc[:], dAf[:],
                        ea_t[:].unsqueeze(1).to_broadcast([128, LCH, NHEADS]))
                    if dbg is not None and dbg.get("mode") == 4:
                        nc.gpsimd.dma_start(dbg["d_dA"].ap(), dAsc[:])
                    # conv(BC) + SiLU evacuation
                    for n in range(NCH):
                        pc = pconv.tile([128, 512], F32, tag="pc")
                        r0 = n * 8
                        for ti in range(9):
                            dy, dx = ti // 3, ti % 3
                            nc.tensor.matmul(
                                pc[:].rearrange("p (r w) -> p r w", w=W_),
                                cdg[:, 18 + ti, :],
                                pads[2][:, r0 + dy:r0 + dy + 8, dx:dx + W_],
                                start=(ti == 0), stop=(ti == 8))
                        nc.scalar.activation(BC[:, n * 512:(n + 1) * 512], pc[:],
                                             AF.Silu, bias=cb[:, 2:3])
                    # B^T via XBAR (overlaps phase 2 compute)
                    nc.sync.dma_start_transpose(Bt[:], BC[0:DSTATE, :])
                    if dbg is not None and dbg.get("mode") == 4:
                        nc.gpsimd.dma_start(dbg["d_Bt"].ap(), Bt[:])

                # ---- Phase 2: xv in_proj + conv + V^T via XBAR + KV ----
                if True:
                    kvp = pkv.tile([DSTATE, DIM], F32)

                    def inproj_xv(n):
                        for g in range(2):
                            ps = p1a.tile([128, 512], F32)
                            for k in range(2):
                                nc.tensor.matmul(ps[:], wxb2[:, k, g * 128:(g + 1) * 128],
                                                 X[:, k, n * 512:(n + 1) * 512],
                                                 start=(k == 0), stop=(k == 1))
                            r0 = n * 8
                            dst = pads[g][:, r0 + 1:r0 + 9, 1:1 + W_]
                            src2 = ps[:].rearrange("p (r w) -> p r w", w=W_)
                            if g == 1:
                                nc.vector.tensor_copy(dst, src2)
                            else:
                                nc.scalar.copy(dst, src2)

                    def conv_v(n):
                        for g in range(2):
                            pc = pconv.tile([128, 512], F32, tag="pc")
                            r0 = n * 8
                            for ti in range(9):
                                dy, dx = ti // 3, ti % 3
                                nc.tensor.matmul(
                                    pc[:].rearrange("p (r w) -> p r w", w=W_),
                                    cdg[:, g * 9 + ti, :],
                                    pads[g][:, r0 + dy:r0 + dy + 8, dx:dx + W_],
                                    start=(ti == 0), stop=(ti == 8))
                            nc.scalar.activation(Vg[g][:, n * 512:(n + 1) * 512], pc[:],
                                                 AF.Silu, bias=cb[:, g:g + 1])
                            eng = nc.sync if g == 0 else nc.scalar
                            eng.dma_start_transpose(
                                Vt[:, 4 * n:4 * n + 4, g * 128:(g + 1) * 128],
                                Vg[g][:, n * 512:(n + 1) * 512])
                        if dbg is not None and dbg.get("mode") == 4:
                            nc.gpsimd.dma_start(dbg["d_Vt"].ap()[:, 4 * n:4 * n + 4, :],
                                                Vt[:, 4 * n:4 * n + 4, :])

                    def z_mm(n):
                        # z in_proj in phase-2 slack; evacs land channel-major
                        # in SBUF so the tail gate can run on the Pool engine
                        for g in range(2):
                            ps = p1a.tile([128, 512], F32)
                            for k in range(2):
                                nc.tensor.matmul(ps[:], wz[:, k, g * 128:(g + 1) * 128],
                                                 X[:, k, n * 512:(n + 1) * 512],
                                                 start=(k == 0), stop=(k == 1))
                            if g == 0:
                                nc.vector.tensor_copy(zcm[:, g, n * 512:(n + 1) * 512], ps[:])
                            else:
                                nc.scalar.copy(zcm[:, g, n * 512:(n + 1) * 512], ps[:])

                    def ssd(n):
                        for i in (4 * n, 4 * n + 2):
                            vst = sbt.tile([128, 2, DIM], BF16, tag="vst")
                            nc.vector.tensor_mul(
                                vst[:].rearrange("p c (h q) -> p c h q", h=NHEADS),
                                Vt[:, i:i + 2, :].rearrange("p c (h q) -> p c h q", h=NHEADS),
                                dAsc[:, i:i + 2, :].unsqueeze(3).to_broadcast(
                                    [128, 2, NHEADS, HEAD_DIM]),
                            )
                            for j in range(2):
                                nc.tensor.matmul(kvp[:], Bt[:, i + j, :], vst[:, j, :],
                                                 start=(i + j == 0), stop=(i + j == LCH - 1))

                    for n in range(NCH + 2):
                        if n < NCH:
                            inproj_xv(n)
                        if 1 <= n < NCH + 1:
                            conv_v(n - 1)
                        if n >= 2:
                            ssd(n - 2)
                            z_mm(n - 2)
                    z_mm(NCH - 2)
                    z_mm(NCH - 1)
                    nc.vector.tensor_copy(kv_sb[DSTATE:2 * DSTATE, :], kvp[:])
                    if dbg is not None and dbg.get("mode") in (1, 4):
                        nc.gpsimd.dma_start(dbg["d_kv"].ap(), kv_sb[DSTATE:2 * DSTATE, :])

            # ---- Phase 3: per-quad pipeline: y, D-skip, LN, XBAR, gate,
            # z in_proj (channel-major, into dead pads), out_proj ----
            with (
                tc.tile_pool(name="pyp", bufs=2, space="PSUM") as pyp,
                tc.tile_pool(name="pzo", bufs=2, space="PSUM") as pzo,
                tc.tile_pool(name="s6", bufs=8) as s6,
                tc.tile_pool(name="s6b", bufs=8) as s6b,
                tc.tile_pool(name="s6g", bufs=12) as s6g,
            ):
                outr = outd.ap().rearrange("(m k) l -> k m l", k=128)
                slabs = [None] * NCH

                def y_quad(q):
                    ypr = pyp.tile([128, 4, DIM], F32, tag="ypr")
                    for j in range(4):
                        i = 4 * q + j
                        sl = slice(i * 128, (i + 1) * 128)
                        nc.tensor.matmul(ypr[:, j, :], BC[DSTATE:2 * DSTATE, sl],
                                         kv_sb[DSTATE:2 * DSTATE, :], start=True, stop=True)
                    # D-skip: single DVE op PSUM->SBUF frees the PSUM tile;
                    # bf16 result halves the DVE time of the bn_stats after
                    ysum = s6b.tile([128, 4, DIM], BF16, tag="ysum")
                    if generic_d:
                        vtmp = s6b.tile([128, 4, DIM], F32, tag="vtmp")
                        nc.vector.tensor_mul(
                            vtmp[:], Vt[:, 4 * q:4 * q + 4, :],
                            dm1_bc[:].unsqueeze(1).to_broadcast([128, 4, DIM]))
                        nc.vector.tensor_add(ysum[:], ypr[:], vtmp[:])
                    else:
                        nc.vector.tensor_add(ysum[:], ypr[:], Vt[:, 4 * q:4 * q + 4, :])
                    st = s6.tile([128, 4, 6], F32, tag="st")
                    for j in range(4):
                        nc.vector.bn_stats(st[:, j, :], ysum[:, j, :])
                    return ysum, st

                def ln_quad(q, ysum, st):
                    # bn_stats groups are the even/odd lanes of each 256-chunk
                    # (st per chunk = [n0, m0, M2_0, n1, m1, M2_1]); merged
                    # with tiny Pool ops:
                    #   mean = (m0+m1)/2
                    #   var  = (M2_0+M2_1)/256 + (m0-m1)^2/4
                    stm = st[:].rearrange("p j (g s) -> p j g s", g=2)
                    msum = s6.tile([128, 4], F32, tag="msum")
                    nc.gpsimd.tensor_add(msum[:], stm[:, :, 0, 1], stm[:, :, 1, 1])
                    mdif = s6.tile([128, 4], F32, tag="mdif")
                    nc.gpsimd.tensor_sub(mdif[:], stm[:, :, 0, 1], stm[:, :, 1, 1])
                    m2s = s6.tile([128, 4], F32, tag="m2s")
                    nc.gpsimd.tensor_add(m2s[:], stm[:, :, 0, 2], stm[:, :, 1, 2])
                    md2 = s6.tile([128, 4], F32, tag="md2")
                    nc.gpsimd.tensor_mul(md2[:], mdif[:], mdif[:])
                    # var*256 = m2s + 64*md2 -> rstd = rsqrt((m2s+64*md2)/256+eps)
                    var256 = s6.tile([128, 4], F32, tag="var256")
                    nc.gpsimd.tensor_scalar_mul(var256[:], md2[:], 64.0)
                    nc.gpsimd.tensor_add(var256[:], var256[:], m2s[:])
                    rstd = s6.tile([128, 4], F32, tag="rstd")
                    nc.scalar.activation(rstd[:], var256[:], AF.Abs_reciprocal_sqrt,
                                         bias=eps_t[:], scale=1.0 / DIM)
                    nmr = s6.tile([128, 4], F32, tag="nmr")
                    nc.gpsimd.tensor_mul(nmr[:], msum[:], rstd[:])
                    nc.gpsimd.tensor_scalar_mul(nmr[:], nmr[:], -0.5)
                    # g-major layout [c? no: l, g, j, c128] so the post-XBAR slab
                    # has each channel group contiguous (fast gate + out stream)
                    yn = s6b.tile([128, 2, 4, 128], BF16, tag="yn")
                    for j in range(4):
                        nc.scalar.activation(
                            yn[:, :, j, :],
                            ysum[:, j, :].rearrange("p (g c) -> p g c", g=2),
                            AF.Identity,
                            bias=nmr[:, j:j + 1], scale=rstd[:, j:j + 1])
                    if has_lnw:
                        nc.vector.tensor_mul(
                            yn[:], yn[:],
                            lnw_bc[:].rearrange("p (g c) -> p g c", g=2)
                            .unsqueeze(2).to_broadcast([128, 2, 4, 128]))
                    if has_lnb:
                        nc.vector.tensor_add(
                            yn[:], yn[:],
                            lnb_bc[:].rearrange("p (g c) -> p g c", g=2)
                            .unsqueeze(2).to_broadcast([128, 2, 4, 128]))
                    return yn

                def tr_quad(q, yn):
                    # slab [c128, g, j, l]: channel-major quad, g contiguous
                    slab = s6g.tile([128, 2, 4, 128], BF16, tag="yzt")
                    nc.sync.dma_start_transpose(
                        slab[:], yn[:].rearrange("a b c d -> a (b c d)"))
                    slabs[q] = slab

                def gate_quad(q):
                    # fully contiguous: slab [c128, g, j, l] vs z [c128, g, 512]
                    slab = slabs[q]
                    zv = zcm[:, :, 512 * q:512 * (q + 1)].rearrange(
                        "p g (n l) -> p g n l", l=128)
                    nc.gpsimd.tensor_mul(slab[:], slab[:], zv)

                def out_quad(q):
                    slab = slabs[q]
                    ob = s6g.tile([128, 2, 512], BF16, tag="ob")
                    po = pzo.tile([128, 2, 512], F32, tag="ops")
                    for mo in range(2):
                        for k in range(2):
                            nc.tensor.matmul(po[:, mo, :], wo[:, k, mo * 128:(mo + 1) * 128],
                                             slab[:, k, :, :], start=(k == 0), stop=(k == 1))
                    # one big evacuation, alternating engines per quad
                    if q % 2 == 0:
                        nc.vector.tensor_copy(ob[:], po[:])
                    else:
                        nc.scalar.copy(ob[:], po[:])
                    nc.scalar.dma_start(outr[:, :, q * 512:(q + 1) * 512], ob[:])

                # software pipeline: PE order y(q), z(q), out(q-2) so out_proj
                # trails the yn/XBAR/gate chain by ~2 quads of slack
                for q in range(NCH + 4):
                    if q < NCH:
                        ysum, st = y_quad(q)
                    if 2 <= q < NCH + 2:
                        gate_quad(q - 2)
                    if q < NCH:
                        yn = ln_quad(q, ysum, st)
                        tr_quad(q, yn)
                    if q >= 4:
                        out_quad(q - 4)


_CACHE = {}


def _prep(W_in, conv_w, conv_b, A_log, dt_bias, D, ln_w, ln_b, W_out):
    W_in = np.asarray(W_in, np.float32)
    conv_w = np.asarray(conv_w, np.float32)
    conv_b = np.asarray(conv_b, np.float32)
    A_log = np.asarray(A_log, np.float32)
    dt_bias = np.asarray(dt_bias, np.float32)
    D = np.asarray(D, np.float32)
    ln_w = np.asarray(ln_w, np.float32)
    ln_b = np.asarray(ln_b, np.float32)
    W_out = np.asarray(W_out, np.float32)

    WinT = np.ascontiguousarray(W_in.T)                       # [256, 648]
    # phase-1 slice: BC cols (W_in rows 512:640) + dt cols (640:648)
    w_ph1 = np.ascontiguousarray(WinT[:, 2 * DIM:]).astype(BF)   # [256, 136]
    w_ph2 = np.ascontiguousarray(WinT[:, DIM:2 * DIM]).astype(BF)  # [256, 256] xv
    w_z = np.ascontiguousarray(WinT[:, :DIM]).astype(BF)      # [256, 256]
    w_out_t = np.ascontiguousarray(W_out.T).astype(BF)        # [256, 256]

    cwm = np.zeros((128, 27), np.float32)
    idx = np.arange(128)
    for g in range(3):
        for t in range(9):
            cwm[:, g * 9 + t] = conv_w[g * 128 + idx, 0, t // 3, t % 3]

    cbm = np.zeros((128, 3), np.float32)
    for g in range(3):
        cbm[:, g] = conv_b[g * 128:(g + 1) * 128]

    flags = {
        "generic_d": not np.allclose(D, 1.0),
        "has_lnw": not np.allclose(ln_w, 1.0),
        "has_lnb": not np.allclose(ln_b, 0.0),
    }
    shared = dict(
        w_ph1=w_ph1, w_ph2=w_ph2, w_z=w_z, w_out=w_out_t,
        identd=np.eye(128, dtype=np.float32).astype(BF), convw=cwm,
        dtbias=dt_bias.reshape(NHEADS, 1).astype(np.float32),
        expa=np.exp(A_log).reshape(NHEADS, 1).astype(np.float32),
        convb=cbm, dexpand=np.repeat(D, HEAD_DIM).reshape(1, DIM).astype(np.float32),
        lnwv=ln_w, lnbv=ln_b,
    )
    return flags, shared


def _get_nc(flags, reps=1):
    key = (tuple(sorted(flags.items())), reps, DUMP_MODE)
    if key not in _CACHE:
        _CACHE[key] = _build(flags, reps)
    return _CACHE[key]


def kernel(x, W_in, conv_w, conv_b, A_log, dt_bias, D, ln_w, ln_b, W_out,
           _trace=False):
    x = np.asarray(x, np.float32)
    flags, shared = _prep(W_in, conv_w, conv_b, A_log, dt_bias, D, ln_w, ln_b, W_out)
    nc = _get_nc(flags)
    xb = x.reshape(B_, DIM, L).astype(BF)
    in_maps = [dict(xb=np.ascontiguousarray(xb[b]), **shared) for b in range(B_)]
    res = run_bass_kernel_spmd(nc, in_maps, core_ids=list(range(NCORES)), trace=_trace)
    out = np.stack([np.asarray(res.results[b]["outb"], np.float32) for b in range(B_)])
    out = out.reshape(B_, DIM, H_, W_)
    if _trace:
        return out, res
    return out
